# revision 1
# baseline (speedup 1.0000x reference)
"""Causal self-attention Bass/TRN2 kernel for nn_CausalSelfAttention.

Shapes (hardcoded): query [2, 2048, 1024], 16 heads, d=64.
Sharding: 8 cores = 2 batches x 4 head-groups (4 heads per core, tensor
parallel on QKV/proj weight columns). Each core computes a partial output
projection out_t = Wp_slice^T @ y^T (shape [1024, 2048]); host sums the 4
partials per batch, transposes, and adds bp.

Per-core pipeline:
  1. PE-transpose X [2048,1024] -> X^T [1024,2048] tiles (fp32 identity mm)
  2. Q^T, K^T = Wq/Wk_slice^T @ X^T (+bias via ACT copy), [256, 2048] f32r
     V = X @ Wv_slice (+bias via K=1 ones matmul), [2048, 256] f32r
  3. Per head-pair: S^T_j = k^T_j.T-style nc_matmul(kT chunk, qT), row-packed
     2 heads via tile_position (0,0)/(64,0); additive -1e30 triangle mask on
     diagonal 128-blocks; ACT exp (scale=1/8, no max-subtraction -- scores
     are bounded |s|<9 for this problem) -> P_j f32r; PV + denominator
     (ones-matmul) col-packed via tile_position (0,0)/(0,64); per-head
     normalization y^T *= 1/den fused on DVE.
  4. out_t = Wp_slice^T @ y^T.

This walrus build accepts only ONE sync-wait command per TPB instruction, so
after Tile scheduling we hoist excess waits into standalone InstEventSemaphore
instructions (split_excess_waits).
"""

import numpy as np

import concourse.bass as bass
import concourse.mybir as mybir
import concourse.tile as tile
from concourse.bass_utils import run_bass_kernel_spmd

B, T, C, H = 2, 2048, 1024, 16
D = C // H            # 64 head dim
HC = 4                # heads per core
DC = HC * D           # 256 dcols per core
KT = C // 128         # 8 contraction tiles
NT = T // 128         # 16 t-tiles
TCH = T // 512        # 4 t-chunks of 512
SCALE = 1.0 / np.sqrt(D)
NEG = -1.0e30

f32 = mybir.dt.float32
f32r = mybir.dt.float32r

_CACHE = {}


def _split_excess_waits(nc, max_inline=1):
    """Hoist excess per-instruction waits into standalone event-sem waits."""
    n = 0
    for f in nc.m.functions:
        for bb in f.blocks:
            new_insts = []
            for inst in bb.instructions:
                si = inst.sync_info
                waits = list(si.on_wait) if (si is not None and si.on_wait) else []
                if len(waits) > max_inline:
                    hoist, keep = waits[:-max_inline], waits[-max_inline:]
                    for w in hoist:
                        ev = mybir.InstEventSemaphore(
                            name=nc.get_next_instruction_name(),
                            engine=inst.engine,
                            ins=[],
                            outs=[],
                            sync_info=mybir.SyncInfo(on_wait=[w], on_update=[]),
                        )
                        nc.register_instruction(ev, overwrite=True)
                        new_insts.append(ev)
                        n += 1
                    si.on_wait = keep
                new_insts.append(inst)
            bb.instructions[:] = new_insts
    return n


def _make_identity(nc, ident):
    # affine_select KEEPS in_ where the predicate holds and writes `fill`
    # where it does not: identity = fill 1.0 where NOT (p - f != 0).
    nc.gpsimd.memset(ident, 0.0)
    nc.gpsimd.affine_select(
        out=ident, in_=ident, compare_op=mybir.AluOpType.not_equal,
        fill=1.0, base=0, pattern=[[-1, 128]], channel_multiplier=1,
    )


def _make_diag_mask(nc, mask):
    """mask[p, f] = 0 where f >= p (valid, t>=s) else -1e30."""
    nc.gpsimd.memset(mask, 0.0)
    nc.gpsimd.affine_select(
        out=mask, in_=mask, compare_op=mybir.AluOpType.is_ge,
        fill=NEG, base=0, pattern=[[1, 128]], channel_multiplier=-1,
    )


def _build_program(debug_dumps=False, stages=4):
    import os as _os
    skip_v = bool(_os.environ.get("SKIP_V"))
    skip_k = bool(_os.environ.get("SKIP_K"))
    skip_q = bool(_os.environ.get("SKIP_Q"))
    nc = bass.Bass("TRN2", target_bir_lowering=False, debug=False)

    x_d = nc.dram_tensor("x", [T, C], f32, kind="ExternalInput").ap()
    wq_d = nc.dram_tensor("wq", [C, DC], f32r, kind="ExternalInput").ap()
    wk_d = nc.dram_tensor("wk", [C, DC], f32r, kind="ExternalInput").ap()
    wv_d = nc.dram_tensor("wv", [C, DC], f32r, kind="ExternalInput").ap()
    wp_d = nc.dram_tensor("wp", [DC, C], f32r, kind="ExternalInput").ap()
    bq_d = nc.dram_tensor("bq", [DC], f32, kind="ExternalInput").ap()
    bk_d = nc.dram_tensor("bk", [DC], f32, kind="ExternalInput").ap()
    bv_d = nc.dram_tensor("bv", [1, DC], f32r, kind="ExternalInput").ap()
    ones_d = nc.dram_tensor("ones_pv", [128, 64], f32r, kind="ExternalInput").ap()
    onesrow_d = nc.dram_tensor("onesrow", [1, 128], f32r, kind="ExternalInput").ap()
    out_d = nc.dram_tensor("out_t", [C, T], f32, kind="ExternalOutput").ap()

    with (
        tile.TileContext(nc) as tc,
        nc.allow_low_precision("float32r is 32-bit storage; rounding is benign"),
    ):
        with (
            tc.tile_pool(name="const", bufs=1) as cpool,
            tc.tile_pool(name="big", bufs=1) as big,
        ):
            # ---- constants ----
            ident = cpool.tile([128, 128], f32)
            _make_identity(nc, ident)
            dmask = cpool.tile([128, 128], f32)
            _make_diag_mask(nc, dmask)
            bq_sb = cpool.tile([128, 2, 1], f32)
            bk_sb = cpool.tile([128, 2, 1], f32)
            for m in range(2):
                nc.sync.dma_start(
                    out=bq_sb[:, m, :],
                    in_=bq_d[bass.ds(128 * m, 128)].rearrange("(p o) -> p o", o=1),
                )
                nc.sync.dma_start(
                    out=bk_sb[:, m, :],
                    in_=bk_d[bass.ds(128 * m, 128)].rearrange("(p o) -> p o", o=1),
                )
            bv_sb = cpool.tile([1, DC], f32r)
            nc.sync.dma_start(out=bv_sb, in_=bv_d)
            ones_pv = cpool.tile([128, 64], f32r)
            nc.sync.dma_start(out=ones_pv, in_=ones_d)
            onesrow = cpool.tile([1, 128], f32r)
            nc.sync.dma_start(out=onesrow, in_=onesrow_d)

            # ---- persistent big tensors ----
            qt = big.tile([128, 2, T], f32r)   # Q^T  [dcol, t]
            kt = big.tile([128, 2, T], f32r)   # K^T
            # V augmented per head: [s, 65] = [V_h | ones]; M=65 PV matmul
            # then computes y rows 0..63 and the softmax denominator row 64.
            va = big.tile([128, HC, NT, 65], f32r)
            yt = big.tile([128, 2, T], f32r)   # normalized y^T

            # ================= stage 1+2: transpose + projections ==========
            with (
                tc.tile_pool(name="xtp", bufs=1) as xtp,
                tc.tile_pool(name="wqk", bufs=1) as wqk,
                tc.tile_pool(name="xn_p", bufs=3) as xn_p,
                tc.tile_pool(name="ps_t", bufs=2, space="PSUM") as ps_t,
                tc.tile_pool(name="ps_qk", bufs=2, space="PSUM") as ps_qk,
                tc.tile_pool(name="ps_v", bufs=2, space="PSUM") as ps_v,
            ):
                xt = xtp.tile([128, KT, T], f32r)  # X^T
                wq_sb = wqk.tile([128, KT, DC], f32r)
                wk_sb = wqk.tile([128, KT, DC], f32r)
                wv_sb = wqk.tile([128, KT, DC], f32r)
                for k in range(KT):
                    nc.sync.dma_start(out=wq_sb[:, k, :], in_=wq_d[bass.ts(k, 128), :])
                    nc.sync.dma_start(out=wk_sb[:, k, :], in_=wk_d[bass.ts(k, 128), :])
                    nc.sync.dma_start(out=wv_sb[:, k, :], in_=wv_d[bass.ts(k, 128), :])

                # transpose X -> X^T; batch 4 transposes per full PSUM bank
                # so no engine ever reads a bank the PE is still writing
                xn_o = None
                if debug_dumps:
                    xn_o = nc.dram_tensor(
                        "xn_o", [128, C], f32, kind="ExternalOutput").ap()
                for it in range(NT):
                    xn = xn_p.tile([128, C], f32)
                    nc.sync.dma_start(out=xn, in_=x_d[bass.ts(it, 128), :])
                    if debug_dumps and it == 0:
                        nc.sync.dma_start(out=xn_o, in_=xn)
                    for kb in range(KT // 4):
                        tp = ps_t.tile([128, 512], f32)
                        for kk in range(4):
                            k = 4 * kb + kk
                            nc.tensor.transpose(
                                tp[:, bass.ts(kk, 128)], xn[:, bass.ts(k, 128)],
                                ident,
                            )
                        nc.vector.tensor_copy(
                            out=xt[:, 4 * kb:4 * kb + 4, bass.ts(it, 128)],
                            in_=tp.rearrange("p (k t) -> p k t", k=4),
                        )

                # Q^T / K^T projections (+bias via ACT copy)
                for m in range(2 if not skip_q else 0):
                    for g in range(TCH):
                        qp = ps_qk.tile([128, 512], f32)
                        for k in range(KT):
                            nc.tensor.matmul(
                                qp,
                                wq_sb[:, k, bass.ts(m, 128)],
                                xt[:, k, bass.ts(g, 512)],
                                start=(k == 0), stop=(k == KT - 1),
                            )
                        nc.scalar.activation(
                            out=qt[:, m, bass.ts(g, 512)], in_=qp,
                            func=mybir.ActivationFunctionType.Identity,
                            bias=bq_sb[:, m, :], scale=1.0,
                        )
                        kp = ps_qk.tile([128, 512], f32)
                        for k in range(KT if not skip_k else 0):
                            nc.tensor.matmul(
                                kp,
                                wk_sb[:, k, bass.ts(m, 128)],
                                xt[:, k, bass.ts(g, 512)],
                                start=(k == 0), stop=(k == KT - 1),
                            )
                        if not skip_k:
                            nc.scalar.activation(
                                out=kt[:, m, bass.ts(g, 512)], in_=kp,
                                func=mybir.ActivationFunctionType.Identity,
                                bias=bk_sb[:, m, :], scale=1.0,
                            )

                # V natural (+bias via K=1 ones matmul)
                if debug_dumps:
                    xt_o = nc.dram_tensor(
                        "xt_o", [128, KT, T], f32, kind="ExternalOutput").ap()
                    wq_o = nc.dram_tensor(
                        "wq_o", [128, KT, DC], f32, kind="ExternalOutput").ap()
                    nc.sync.dma_start(out=xt_o, in_=xt.bitcast(f32))
                    nc.sync.dma_start(out=wq_o, in_=wq_sb.bitcast(f32))

                for it in range(NT if not skip_v else 0):
                    # full-bank allocation (use first DC cols) to avoid
                    # intra-bank PE-write / DVE-read overlap
                    vp_full = ps_v.tile([128, 512], f32)
                    vp = vp_full[:, 0:DC]
                    for k in range(KT):
                        nc.tensor.matmul(
                            vp,
                            xt[:, k, bass.ts(it, 128)],
                            wv_sb[:, k, :],
                            start=(k == 0), stop=False,
                        )
                    import os as _os
                    if not _os.environ.get("SKIP_BV"):
                        nc.tensor.matmul(
                            vp, onesrow, bv_sb, start=False, stop=True,
                        )
                    else:
                        pass
                    for h in range(HC):
                        nc.vector.tensor_copy(
                            out=va[:, h, it, 0:64], in_=vp[:, bass.ts(h, 64)]
                        )
                # ones column of each v_aug
                for h in range(HC):
                    nc.vector.tensor_copy(
                        out=va[:, h, :, 64:65],
                        in_=ones_pv[:, 0:NT].rearrange("p (n o) -> p n o", o=1),
                    )

            # ================= stage 3: attention =========================
            def attention_headpair(hp, pools, after_g=None):
                pp, den_p, ps_s, ps_y, ps_b = pools  # ps_b aliases ps_o
                h1, h2 = 2 * hp, 2 * hp + 1
                for g in range(TCH):
                    yd1 = ps_y.tile([128, 512], f32, name="yd1")
                    yd2 = ps_y.tile([128, 512], f32, name="yd2")
                    nj = 4 * g + 4
                    for j in range(nj):
                        r = j - 4 * g
                        lo = 128 * r if r > 0 else 0
                        w = 512 - lo
                        # both heads' S^T in one 2-bank psum tile
                        s12 = ps_s.tile([128, 1024], f32, name="s12")
                        tsl = bass.ds(512 * g + lo, w)
                        nc.tensor.matmul(
                            s12[:, lo:512], kt[0:64, hp, bass.ts(j, 128)],
                            qt[0:64, hp, tsl], start=True, stop=True,
                        )
                        nc.tensor.matmul(
                            s12[:, 512 + lo:1024], kt[64:128, hp, bass.ts(j, 128)],
                            qt[64:128, hp, tsl], start=True, stop=True,
                        )
                        if r >= 0:
                            nc.vector.tensor_add(
                                s12[:, lo:lo + 128], s12[:, lo:lo + 128], dmask
                            )
                            nc.vector.tensor_add(
                                s12[:, 512 + lo:512 + lo + 128],
                                s12[:, 512 + lo:512 + lo + 128], dmask
                            )
                        p12 = pp.tile([128, 1024], f32r, name="p12")
                        sv = s12.rearrange("p (h t) -> p h t", h=2)[:, :, lo:]
                        pv = p12.rearrange("p (h t) -> p h t", h=2)[:, :, lo:]
                        nc.scalar.activation(
                            out=pv, in_=sv,
                            func=mybir.ActivationFunctionType.Exp,
                            scale=float(SCALE),
                        )
                        last = j == nj - 1
                        nc.tensor.matmul(
                            yd1[0:65, lo:], va[:, h1 % 4, j, :],
                            p12[:, lo:512], start=(j == 0), stop=last,
                            skip_group_check=True,
                        )
                        nc.tensor.matmul(
                            yd2[0:65, lo:], va[:, h2 % 4, j, :],
                            p12[:, 512 + lo:1024], start=(j == 0), stop=last,
                            skip_group_check=True,
                        )
                    # normalize: recip of den row 64, broadcast to 64 rows
                    # via ones matmul, multiply into y rows
                    for odd, yd in ((0, yd1), (1, yd2)):
                        r1 = den_p.tile([128, 512], f32r, name="r1")
                        nc.vector.reciprocal(
                            out=r1[64:65, :], in_=yd[64:65, :]
                        )
                        # K=1 matmul with lhsT/rhs at partition 64 (row
                        # group (64,0)): broadcasts 1/den to 64 rows without
                        # a partition-move DMA in the critical chain
                        bc = ps_b.tile([128, 512], f32, name="op")[0:64, :]
                        nc.tensor.matmul(
                            bc, ones_pv[64:65, :], r1[64:65, :],
                            start=True, stop=True,
                        )
                        rb = den_p.tile([64, 512], f32, name="rb")
                        nc.vector.tensor_copy(out=rb, in_=bc)
                        if odd == 0:
                            nc.vector.tensor_mul(
                                yt[0:64, hp, bass.ts(g, 512)], yd[0:64, :], rb
                            )
                        else:
                            ytmp = den_p.tile([64, 512], f32r, name="ytmp")
                            nc.vector.tensor_mul(ytmp, yd[0:64, :], rb)
                            nc.sync.dma_start(
                                out=yt[64:128, hp, bass.ts(g, 512)], in_=ytmp,
                            )
                    if after_g is not None:
                        after_g(g)

            if stages >= 3:
                with (
                    tc.tile_pool(name="pp", bufs=4) as pp,
                    tc.tile_pool(name="den_p", bufs=2) as den_p,
                    tc.tile_pool(name="wpp", bufs=1) as wpp,
                    tc.tile_pool(name="ob_p", bufs=3) as ob_p,
                    tc.tile_pool(name="ps_s", bufs=2, space="PSUM") as ps_s,
                    tc.tile_pool(name="ps_y", bufs=1, space="PSUM") as ps_y,
                    tc.tile_pool(name="ps_o", bufs=2, space="PSUM") as ps_o,
                ):
                    wp_sb = wpp.tile([128, 2, 8, 128], f32r)
                    for m in range(2):
                        for mo in range(8):
                            nc.sync.dma_start(
                                out=wp_sb[:, m, mo, :],
                                in_=wp_d[bass.ts(m, 128), bass.ts(mo, 128)],
                            )

                    def outproj_g(g):
                        for mo in range(8):
                            op = ps_o.tile([128, 512], f32, name="op")
                            for m in range(2):
                                nc.tensor.matmul(
                                    op, wp_sb[:, m, mo, :],
                                    yt[:, m, bass.ts(g, 512)],
                                    start=(m == 0), stop=(m == 1),
                                )
                            ob = ob_p.tile([128, 512], f32, name="ob")
                            nc.vector.tensor_copy(out=ob, in_=op)
                            nc.sync.dma_start(
                                out=out_d[bass.ts(mo, 128), bass.ts(g, 512)],
                                in_=ob,
                            )

                    pools = (pp, den_p, ps_s, ps_y, ps_o)
                    attention_headpair(0, pools)
                    attention_headpair(1, pools, after_g=outproj_g)

            if debug_dumps:
                qt_o = nc.dram_tensor(
                    "qt_o", [128, 2, T], f32, kind="ExternalOutput").ap()
                kt_o = nc.dram_tensor(
                    "kt_o", [128, 2, T], f32, kind="ExternalOutput").ap()
                va_o = nc.dram_tensor(
                    "va_o", [128, HC, NT, 65], f32, kind="ExternalOutput").ap()
                yt_o = nc.dram_tensor(
                    "yt_o", [128, 2, T], f32, kind="ExternalOutput").ap()
                if not skip_q:
                    nc.sync.dma_start(out=qt_o, in_=qt.bitcast(f32))
                if not skip_k:
                    nc.sync.dma_start(out=kt_o, in_=kt.bitcast(f32))
                if not skip_v:
                    nc.sync.dma_start(out=va_o, in_=va.bitcast(f32))
                if stages >= 3:
                    nc.sync.dma_start(out=yt_o, in_=yt.bitcast(f32))

    _split_excess_waits(nc)
    return nc


def kernel(**inputs) -> np.ndarray:
    query = np.ascontiguousarray(np.asarray(inputs["query"], dtype=np.float32))
    Wq = np.asarray(inputs["Wq"], dtype=np.float32)
    Wk = np.asarray(inputs["Wk"], dtype=np.float32)
    Wv = np.asarray(inputs["Wv"], dtype=np.float32)
    Wp = np.asarray(inputs["Wp"], dtype=np.float32)
    bq = np.asarray(inputs["bq"], dtype=np.float32)
    bk = np.asarray(inputs["bk"], dtype=np.float32)
    bv = np.asarray(inputs["bv"], dtype=np.float32)
    bp = np.asarray(inputs["bp"], dtype=np.float32)
    n_head = int(inputs.get("n_head", H))
    assert n_head == H, f"kernel hardcodes n_head={H}, got {n_head}"
    assert query.shape == (B, T, C)

    if "nc" not in _CACHE:
        _CACHE["nc"] = _build_program()
    nc = _CACHE["nc"]

    ones_pv = np.ones((128, 64), np.float32)
    onesrow = np.ones((1, 128), np.float32)
    in_maps = []
    for c in range(8):
        b = c // 4
        hg = c % 4
        cols = slice(DC * hg, DC * (hg + 1))
        in_maps.append({
            "x": query[b],
            "wq": np.ascontiguousarray(Wq[:, cols]),
            "wk": np.ascontiguousarray(Wk[:, cols]),
            "wv": np.ascontiguousarray(Wv[:, cols]),
            "wp": np.ascontiguousarray(Wp[cols, :]),
            "bq": np.ascontiguousarray(bq[cols]),
            "bk": np.ascontiguousarray(bk[cols]),
            "bv": np.ascontiguousarray(bv[cols])[None, :],
            "ones_pv": ones_pv,
            "onesrow": onesrow,
        })

    res = run_bass_kernel_spmd(nc, in_maps, core_ids=list(range(8)))
    _CACHE["last_res"] = res

    out = np.empty((B, T, C), np.float32)
    for b in range(B):
        acc = res.results[4 * b]["out_t"].astype(np.float32)
        for c in range(4 * b + 1, 4 * b + 4):
            acc = acc + res.results[c]["out_t"]
        out[b] = acc.T + bp
    return out



# revision 6
# speedup vs baseline: 1.1553x; 1.1553x over previous
"""Causal self-attention Bass/TRN2 kernel (v2, bf16 compute).

Shapes (hardcoded): query [2, 2048, 1024], 16 heads, d=64.
Sharding: 8 cores = 2 batches x 4 head-groups (4 heads per core, tensor
parallel on the QKV/proj weight columns). Each core computes a partial
out projection out_t = Wp_slice^T @ y^T (shape [1024, 2048] f32); host sums
the 4 partials per batch, transposes, and adds the folded bias.

Host-side exact simplifications:
  * x is pre-transposed per batch (x^T [1024, 2048]) and cast to bf16, so the
    device never runs PE transposes.
  * bk is dropped: q . bk is constant along the softmax axis (shift
    invariance), so it never affects the output.
  * bv is folded into the output bias: y = P@(x Wv) + (P@1) bv^T and softmax
    rows sum to 1 after normalization, so out += bv @ Wp, added to bp on host.
  * bq is applied on device (fused into the Q PSUM->SBUF activation copy).

Per-core pipeline (all matmuls bf16, f32 PSUM accumulate):
  B(m,g): Q^T/K^T [128, 512]-chunk projections (8 k-matmuls each) + ACT copy
          (Q with bq bias) -> qt/kt bf16 [128, 2, 2048].
  C(it):  V natural [128, 256] (8 k-matmuls) + DVE copy into va bf16
          [128, h, it, 65]; column 64 is memset to 1 so the M=65 PV matmul
          also produces the softmax denominator row.
  D(hp,g): per 128-row K-block j: S^T for both heads of the pair into one
          [128, 1024] PSUM tile; additive -1e30 causal mask on the diagonal
          128-blocks (DVE); ACT exp (scale=1/8, no max subtraction -- scores
          are bounded for this problem) -> p12 bf16; PV accumulate into
          yd1/yd2 [65, 512] PSUM. Emission is software-pipelined 2 blocks
          ahead so ACT exp latency never stalls the PE. Normalization:
          DVE reciprocal of the denominator row, Pool partition_broadcast,
          DVE multiply -> yt bf16 (head 1 of the pair lands at partitions
          64:128 via a small SBUF->SBUF shift DMA).
  E(g):   out_t chunk = Wp^T y^T, staged PSUM->SBUF on alternating DVE/ACT,
          DMA out f32.
Schedule interleaves B/C/E between D chunks to keep the PE saturated while
the ACT engine drains the exp backlog.

This walrus build accepts only ONE sync-wait command per TPB instruction, so
after Tile scheduling we hoist excess waits into standalone InstEventSemaphore
instructions (split_excess_waits).
"""

import numpy as np
import ml_dtypes

import concourse.bass as bass
import concourse.mybir as mybir
import concourse.tile as tile
from concourse.bass_utils import run_bass_kernel_spmd

B, T, C, H = 2, 2048, 1024, 16
D = C // H            # 64 head dim
HC = 4                # heads per core
DC = HC * D           # 256 dcols per core
KT = C // 128         # 8 contraction tiles
NT = T // 128         # 16 t-tiles
TCH = T // 512        # 4 t-chunks of 512
SCALE = 1.0 / np.sqrt(D)
NEG = -1.0e30

f32 = mybir.dt.float32
f32r = mybir.dt.float32r
bf16 = mybir.dt.bfloat16
BF = ml_dtypes.bfloat16

_CACHE = {}


def _split_excess_waits(nc, max_inline=1):
    """Hoist excess per-instruction waits into standalone event-sem waits."""
    n = 0
    for f in nc.m.functions:
        for bb in f.blocks:
            new_insts = []
            for inst in bb.instructions:
                si = inst.sync_info
                waits = list(si.on_wait) if (si is not None and si.on_wait) else []
                if len(waits) > max_inline:
                    hoist, keep = waits[:-max_inline], waits[-max_inline:]
                    for w in hoist:
                        ev = mybir.InstEventSemaphore(
                            name=nc.get_next_instruction_name(),
                            engine=inst.engine,
                            ins=[],
                            outs=[],
                            sync_info=mybir.SyncInfo(on_wait=[w], on_update=[]),
                        )
                        nc.register_instruction(ev, overwrite=True)
                        new_insts.append(ev)
                        n += 1
                    si.on_wait = keep
                new_insts.append(inst)
            bb.instructions[:] = new_insts
    return n


def _make_diag_mask(nc, mask):
    """mask[p, f] = 0 where f >= p (valid, t>=s) else -1e30."""
    nc.gpsimd.memset(mask, 0.0)
    nc.gpsimd.affine_select(
        out=mask, in_=mask, compare_op=mybir.AluOpType.is_ge,
        fill=NEG, base=0, pattern=[[1, 128]], channel_multiplier=-1,
    )


def _build_program():
    nc = bass.Bass("TRN2", target_bir_lowering=False, debug=False)

    xt_d = nc.dram_tensor("xt", [C, T], bf16, kind="ExternalInput").ap()
    wq_d = nc.dram_tensor("wq", [C, DC], bf16, kind="ExternalInput").ap()
    wk_d = nc.dram_tensor("wk", [C, DC], bf16, kind="ExternalInput").ap()
    wv_d = nc.dram_tensor("wv", [C, DC], bf16, kind="ExternalInput").ap()
    wp_d = nc.dram_tensor("wp", [DC, C], bf16, kind="ExternalInput").ap()
    bq_d = nc.dram_tensor("bq", [DC], f32, kind="ExternalInput").ap()
    out_d = nc.dram_tensor("out_t", [C, T], f32, kind="ExternalOutput").ap()

    ident_fn = mybir.ActivationFunctionType.Identity

    with (
        tile.TileContext(nc) as tc,
        nc.allow_low_precision("bf16 compute fits the 2e-2 rel tolerance"),
    ):
        with (
            tc.tile_pool(name="const", bufs=1) as cpool,
            tc.tile_pool(name="big", bufs=1) as big,
            tc.tile_pool(name="pp", bufs=4) as pp,
            tc.tile_pool(name="rp", bufs=2) as rp,
            tc.tile_pool(name="rbp", bufs=2) as rbp,
            tc.tile_pool(name="ytp", bufs=2) as ytp,
            tc.tile_pool(name="obp", bufs=3) as obp,
            tc.tile_pool(name="ps_ay", bufs=2, space="PSUM") as ps_ay,
            tc.tile_pool(name="ps_s", bufs=3, space="PSUM") as ps_s,
        ):
            # All [128, 512] f32 PSUM scratch shares one 2-bank rotation
            # ("acc"): qp/kp/vp/yd1/yd2/op lifetimes are strictly sequential
            # in the serial schedule below. s12 gets 3 x 2 banks. Total 8.
            def acc_tile():
                return ps_ay.tile([128, 512], f32, name="acc")
            # ---- constants ----
            dmask = cpool.tile([128, 128], f32)
            _make_diag_mask(nc, dmask)
            bq_sb = cpool.tile([128, 2, 1], f32)
            ones_f = cpool.tile([128, 64], f32)
            nc.gpsimd.memset(ones_f, 1.0)
            ones_sb = ones_f.bitcast(f32r)

            # ---- persistent big tensors ----
            xt = big.tile([128, KT, T], bf16)      # X^T
            wq_sb = big.tile([128, KT, DC], bf16)
            wk_sb = big.tile([128, KT, DC], bf16)
            wv_sb = big.tile([128, KT, DC], bf16)
            wp_sb = big.tile([128, 2, C], bf16)
            qt = big.tile([128, 2, T], bf16)       # Q^T [dcol, t]
            kt = big.tile([128, 2, T], bf16)       # K^T
            # V augmented per head: [s, 65] = [V_h | ones]; the M=65 PV matmul
            # computes y rows 0..63 and the softmax denominator row 64.
            va = big.tile([128, HC, NT, 65], bf16)
            yt = big.tile([128, 2, T], bf16)       # normalized y^T

            nc.gpsimd.memset(va[:, :, :, 64:65], 1.0)

            # ---- input DMAs, ordered for earliest PE start ----
            nc.sync.dma_start(
                out=bq_sb, in_=bq_d.rearrange("(m p o) -> p m o", p=128, o=1))
            nc.sync.dma_start(
                out=wq_sb, in_=wq_d.rearrange("(k p) d -> p k d", p=128))
            nc.sync.dma_start(
                out=xt[:, :, 0:512],
                in_=xt_d[:, 0:512].rearrange("(k p) t -> p k t", p=128))
            nc.sync.dma_start(
                out=wv_sb, in_=wv_d.rearrange("(k p) d -> p k d", p=128))
            nc.sync.dma_start(
                out=wk_sb, in_=wk_d.rearrange("(k p) d -> p k d", p=128))
            for g in range(1, TCH):
                nc.sync.dma_start(
                    out=xt[:, :, bass.ts(g, 512)],
                    in_=xt_d[:, bass.ts(g, 512)].rearrange(
                        "(k p) t -> p k t", p=128))
            nc.sync.dma_start(
                out=wp_sb, in_=wp_d.rearrange("(m p) c -> p m c", p=128))

            # ---- stage helpers ----
            def proj_qk(m, g):
                ts_g = bass.ts(g, 512)
                qp = acc_tile()
                for k in range(KT):
                    nc.tensor.matmul(
                        qp, wq_sb[:, k, bass.ts(m, 128)], xt[:, k, ts_g],
                        start=(k == 0), stop=(k == KT - 1),
                    )
                nc.scalar.activation(
                    out=qt[:, m, ts_g], in_=qp, func=ident_fn,
                    bias=bq_sb[:, m, :], scale=1.0,
                )
                kp = acc_tile()
                for k in range(KT):
                    nc.tensor.matmul(
                        kp, wk_sb[:, k, bass.ts(m, 128)], xt[:, k, ts_g],
                        start=(k == 0), stop=(k == KT - 1),
                    )
                nc.scalar.copy(out=kt[:, m, ts_g], in_=kp)

            def proj_v(it):
                # full-bank allocation (use first DC cols) to avoid
                # intra-bank PE-write / DVE-read overlap
                vp_full = acc_tile()
                vp = vp_full[:, 0:DC]
                for k in range(KT):
                    nc.tensor.matmul(
                        vp, xt[:, k, bass.ts(it, 128)], wv_sb[:, k, :],
                        start=(k == 0), stop=(k == KT - 1),
                    )
                nc.vector.tensor_copy(
                    out=va[:, :, it, 0:64],
                    in_=vp.rearrange("p (h d) -> p h d", h=HC),
                )

            def attn(hp, g):
                nj = 4 * g + 4
                yd1 = acc_tile()
                yd2 = acc_tile()

                def emit_s(j):
                    r = j - 4 * g
                    lo = 128 * r if r > 0 else 0
                    w = 512 - lo
                    s12 = ps_s.tile([128, 1024], f32, name="s12")
                    tsl = bass.ds(512 * g + lo, w)
                    nc.tensor.matmul(
                        s12[:, lo:512], kt[0:64, hp, bass.ts(j, 128)],
                        qt[0:64, hp, tsl], start=True, stop=True,
                    )
                    nc.tensor.matmul(
                        s12[:, 512 + lo:1024], kt[64:128, hp, bass.ts(j, 128)],
                        qt[64:128, hp, tsl], start=True, stop=True,
                    )
                    if r >= 0:
                        nc.vector.tensor_add(
                            s12[:, lo:lo + 128], s12[:, lo:lo + 128], dmask)
                        nc.vector.tensor_add(
                            s12[:, 512 + lo:512 + lo + 128],
                            s12[:, 512 + lo:512 + lo + 128], dmask)
                    p12 = pp.tile([128, 1024], bf16, name="p12")
                    sv = s12.rearrange("p (h t) -> p h t", h=2)[:, :, lo:]
                    pv_ = p12.rearrange("p (h t) -> p h t", h=2)[:, :, lo:]
                    nc.scalar.activation(
                        out=pv_, in_=sv,
                        func=mybir.ActivationFunctionType.Exp,
                        scale=float(SCALE),
                    )
                    return (j, p12, lo)

                def emit_pv(j, p12, lo):
                    last = j == nj - 1
                    nc.tensor.matmul(
                        yd1[0:65, lo:], va[:, (2 * hp) % 4, j, :],
                        p12[:, lo:512], start=(j == 0), stop=last,
                        skip_group_check=True,
                    )
                    nc.tensor.matmul(
                        yd2[0:65, lo:], va[:, (2 * hp + 1) % 4, j, :],
                        p12[:, 512 + lo:1024], start=(j == 0), stop=last,
                        skip_group_check=True,
                    )

                # software pipeline: PV lags S/exp by 2 blocks so ACT exp
                # latency stays off the PE critical path
                pend = []
                for j in range(nj):
                    pend.append(emit_s(j))
                    if len(pend) > 2:
                        emit_pv(*pend.pop(0))
                while pend:
                    emit_pv(*pend.pop(0))

                # normalize: 1/den (row 64), broadcast to 64 rows via a
                # K=1 ones matmul at row group (64,0) (walrus rejects
                # InstPartitionBroadcast), multiply into yt. The broadcast
                # PSUM tile borrows the s12 rotation slot.
                for head, yd in ((0, yd1), (1, yd2)):
                    r1 = rp.tile([128, 512], f32r, name="r1")
                    nc.vector.reciprocal(out=r1[64:65, :], in_=yd[64:65, :])
                    bct = ps_s.tile([128, 1024], f32, name="s12")
                    bc = bct[0:64, 0:512]
                    nc.tensor.matmul(
                        bc, ones_sb[64:65, :], r1[64:65, :],
                        start=True, stop=True,
                    )
                    rb = rbp.tile([64, 512], f32, name="rb")
                    nc.vector.tensor_copy(out=rb, in_=bc)
                    if head == 0:
                        nc.vector.tensor_mul(
                            yt[0:64, hp, bass.ts(g, 512)], yd[0:64, :], rb)
                    else:
                        ytmp = ytp.tile([64, 512], bf16, name="ytmp")
                        nc.vector.tensor_mul(ytmp, yd[0:64, :], rb)
                        nc.sync.dma_start(
                            out=yt[64:128, hp, bass.ts(g, 512)], in_=ytmp)

            def outproj(g):
                for mo in range(8):
                    op = acc_tile()
                    for m in range(2):
                        nc.tensor.matmul(
                            op, wp_sb[:, m, bass.ts(mo, 128)],
                            yt[:, m, bass.ts(g, 512)],
                            start=(m == 0), stop=(m == 1),
                        )
                    ob = obp.tile([128, 512], f32, name="ob")
                    if mo % 2 == 0:
                        nc.vector.tensor_copy(out=ob, in_=op)
                    else:
                        nc.scalar.copy(out=ob, in_=op)
                    nc.sync.dma_start(
                        out=out_d[bass.ts(mo, 128), bass.ts(g, 512)], in_=ob)

            # ---- emission schedule ----
            # serial B+C, then attention with out-projection interleaved one
            # chunk behind (E(g) needs both head-pairs' yt for chunk g, and
            # lagging a chunk keeps the finalize DMA chain off the PE path)
            for g in range(TCH):
                proj_qk(0, g)
                proj_qk(1, g)
                for it in range(4 * g, 4 * g + 4):
                    proj_v(it)
            for g in range(TCH):
                attn(0, g)
            attn(1, 0)
            attn(1, 1)
            outproj(0)
            attn(1, 2)
            outproj(1)
            attn(1, 3)
            outproj(2)
            outproj(3)

    _split_excess_waits(nc)
    return nc


def kernel(**inputs) -> np.ndarray:
    query = np.ascontiguousarray(np.asarray(inputs["query"], dtype=np.float32))
    Wq = np.asarray(inputs["Wq"], dtype=np.float32)
    Wk = np.asarray(inputs["Wk"], dtype=np.float32)
    Wv = np.asarray(inputs["Wv"], dtype=np.float32)
    Wp = np.asarray(inputs["Wp"], dtype=np.float32)
    bq = np.asarray(inputs["bq"], dtype=np.float32)
    bk = np.asarray(inputs["bk"], dtype=np.float32)  # noqa: F841 (exactly dropped)
    bv = np.asarray(inputs["bv"], dtype=np.float32)
    bp = np.asarray(inputs["bp"], dtype=np.float32)
    n_head = int(inputs.get("n_head", H))
    assert n_head == H, f"kernel hardcodes n_head={H}, got {n_head}"
    assert query.shape == (B, T, C)

    if "nc" not in _CACHE:
        _CACHE["nc"] = _build_program()
    nc = _CACHE["nc"]

    # bv contributes bv @ Wp to every output row (softmax rows sum to 1)
    bp_eff = bp + bv @ Wp

    xt_b = [np.ascontiguousarray(query[b].T).astype(BF) for b in range(B)]
    in_maps = []
    for c in range(8):
        b, hg = divmod(c, 4)
        cols = slice(DC * hg, DC * (hg + 1))
        in_maps.append({
            "xt": xt_b[b],
            "wq": np.ascontiguousarray(Wq[:, cols]).astype(BF),
            "wk": np.ascontiguousarray(Wk[:, cols]).astype(BF),
            "wv": np.ascontiguousarray(Wv[:, cols]).astype(BF),
            "wp": np.ascontiguousarray(Wp[cols, :]).astype(BF),
            "bq": np.ascontiguousarray(bq[cols]),
        })

    res = run_bass_kernel_spmd(nc, in_maps, core_ids=list(range(8)))
    _CACHE["last_res"] = res

    out = np.empty((B, T, C), np.float32)
    for b in range(B):
        acc = res.results[4 * b]["out_t"].astype(np.float32)
        for c in range(4 * b + 1, 4 * b + 4):
            acc = acc + res.results[c]["out_t"]
        out[b] = acc.T + bp_eff
    return out
